# revision 1
# baseline (speedup 1.0000x reference)
"""Trainium2 Bass kernel: bidirectional-LSTM final-cell-state encoder.

Problem: 4 independent BasicLSTMCell chains (premise fw/bw, hypothesis fw/bw),
B=1024, T=128, D=300, H=100.  Output [B, 4H] = concat of final cell states.

Strategy (v5)
-------------
* Data-parallel: batch sharded 8 ways -> 128 rows/core; each core runs the 4
  chains as independent per-run recurrences (the per-step latency of one
  chain is the whole kernel's period, so the chain is kept as short as
  possible and the four chains hide each other's engine time).
* z_t [128b, 400g] accumulated in PSUM per run (1 bank each, double
  buffered = 8 banks) from 3 pre-transposed x chunks + 1 recurrent h^T chunk
  (all bf16).  x chunks stream from DRAM with one-block DMA prefetch.
* Gate columns permuted host-side to (i, f, o, j) and the j columns scaled
  by 2 so ONE sigmoid activation per run covers all four gates:
  tanh(zj) == 2*sigmoid(2*zj) - 1.
* Cell state kept as c' = c/2 so the tanh identity constants fold away:
    P' = (sig(2zj) - 0.5) * sig(zi)      [DVE scalar_tensor_tensor]
    F  = sig(zf) * c'                    [GpSimd tensor_tensor]
    c2' = F + P'                         [DVE tensor_tensor]
    tc = tanh(2 * c2')                   [ACT, scale=2]
    h  = tc * sig(zo)                    [DVE tensor_tensor]
  Host multiplies the gathered output by 2.
* Engine assignment keeps every instruction's producers on a single engine
  (TRN2 instructions embed only one semaphore wait; multi-engine fan-in
  costs an extra EVENT_SEMAPHORE).  S and HH are double-buffered to turn
  cross-engine WAR hazards into already-satisfied waits.
* h^T: bf16 PE transpose into dead PSUM columns of the z tile, evacuated by
  DVE; transpose+evac of step t-1 are issued between step t's x-matmuls and
  h-matmuls so the PE FIFO never parks a dependent transpose in front of
  independent x-projection work.
"""

import numpy as np

B, T, D, H = 1024, 128, 300, 100
NCORES = 8
BL = B // NCORES          # 128 batch rows per core
G4 = 4 * H                # 400 gate columns
KCH = 128                 # 100 d-rows + 1 ones-row + zero-pad (FWL needs K=128)
TB = 8                    # timesteps per DMA block
FORGET_BIAS = 1.0

_CACHE = {}


def _build_program(n_steps=T):
    from contextlib import ExitStack

    import concourse.mybir as mybir
    import concourse.tile as tile
    from concourse import bacc

    f32 = mybir.dt.float32
    bf16 = mybir.dt.bfloat16
    Sig = mybir.ActivationFunctionType.Sigmoid
    Tanh = mybir.ActivationFunctionType.Tanh
    mult = mybir.AluOpType.mult
    add = mybir.AluOpType.add

    nc = bacc.Bacc(
        "TRN2",
        target_bir_lowering=False,
        debug=False,
        enable_asserts=False,
        num_devices=NCORES,
    )

    xt_p = nc.dram_tensor("xt_p", [T // TB, KCH, TB * 3 * BL], bf16, kind="ExternalInput").ap()
    xt_h = nc.dram_tensor("xt_h", [T // TB, KCH, TB * 3 * BL], bf16, kind="ExternalInput").ap()
    w_all = nc.dram_tensor("w_all", [KCH, 16 * G4], bf16, kind="ExternalInput").ap()
    wh_bf = nc.dram_tensor("wh_bf", [128, 4 * G4], bf16, kind="ExternalInput").ap()
    ident = nc.dram_tensor("ident", [128, 128], bf16, kind="ExternalInput").ap()
    out = nc.dram_tensor("out", [BL, G4], f32, kind="ExternalOutput").ap()

    with tile.TileContext(nc) as tc, ExitStack() as ctx:
        w_sb = nc.alloc_sbuf_tensor("w_sb", [KCH, 16 * G4], bf16).ap()
        wh_sb = nc.alloc_sbuf_tensor("wh_sb", [128, 4 * G4], bf16).ap()
        id_sb = nc.alloc_sbuf_tensor("id_sb", [128, 128], bf16).ap()

        # per-pair tensors; S and HH double-buffered (cross-engine WAR)
        S, PP, FF, CC, TC, HH = [], [], [], [], [], []
        for p in range(2):
            S.append([nc.alloc_sbuf_tensor(f"s{p}_{b}", [BL, 800], bf16).ap() for b in range(2)])
            PP.append(nc.alloc_sbuf_tensor(f"pp{p}", [BL, 200], bf16).ap())
            FF.append(nc.alloc_sbuf_tensor(f"ff{p}", [BL, 200], f32).ap())
            CC.append(nc.alloc_sbuf_tensor(f"cc{p}", [BL, 200], f32).ap())
            TC.append(nc.alloc_sbuf_tensor(f"tc{p}", [BL, 200], bf16).ap())
            HH.append([nc.alloc_sbuf_tensor(f"hh{p}_{b}", [BL, 200], bf16).ap() for b in range(2)])
        HT = [nc.alloc_sbuf_tensor(f"ht{p}", [128, 256], bf16).ap() for p in range(2)]

        nc.gpsimd.dma_start(w_sb, w_all)
        nc.gpsimd.dma_start(wh_sb, wh_bf)
        nc.gpsimd.dma_start(id_sb, ident)
        for p in range(2):
            nc.vector.memset(CC[p], 0.0)
        for p in range(2):
            nc.vector.memset(HT[p], 0.0)

        xt_pools = [
            ctx.enter_context(tc.tile_pool(name=f"xt{s}", bufs=2)) for s in range(4)
        ]
        zpools = [
            ctx.enter_context(tc.tile_pool(name=f"zp{p}", bufs=2, space="PSUM"))
            for p in range(2)
        ]

        # stream s: (dram tensor, reversed?) for runs (p_fw, p_bw, h_fw, h_bw)
        streams = [(xt_p, False), (xt_p, True), (xt_h, False), (xt_h, True)]
        cur = [None] * 4
        nxt = [None] * 4

        def dma_block(bi, into):
            for s, (dram, rev) in enumerate(streams):
                tl = xt_pools[s].tile(
                    [KCH, TB * 3 * 128], bf16, tag=f"x{s}", name=f"x{s}_b{bi}"
                )
                nblk = (T // TB - 1 - bi) if rev else bi
                nc.sync.dma_start(tl[:, :], dram[nblk])
                into[s] = tl

        dma_block(0, cur)

        prev_z = None  # previous step's z tiles (for deferred transpose+evac)

        for t in range(n_steps):
            if t % TB == 0:
                if t > 0:
                    cur, nxt = nxt, [None] * 4
                if t + TB < n_steps:
                    dma_block(t // TB + 1, nxt)
            sb = t % 2   # S/HH buffer parity

            z = [
                zpools[p].tile([BL, 1024], f32, tag=f"z{p}", name=f"z{p}_{t}")
                for p in range(2)
            ]
            # x-projection matmuls first: independent of the recurrence, so
            # they fill the PE while the previous step's elementwise chain runs
            for r in range(4):
                p, rh = divmod(r, 2)
                rev = streams[r][1]
                tq = (TB - 1 - t % TB) if rev else (t % TB)
                tl = cur[r]
                for k in range(3):
                    nc.tensor.matmul(
                        z[p][:, rh * 512 : rh * 512 + G4],
                        tl[:, (tq * 3 + k) * 128 : (tq * 3 + k + 1) * 128],
                        w_sb[:, (r * 4 + k) * G4 : (r * 4 + k + 1) * G4],
                        start=(k == 0),
                        stop=False,
                    )
            if prev_z is not None:
                # transpose h(t-1) into dead PSUM cols + evac to SBUF, placed
                # here so they sit between x(t) and h(t) in the PE FIFO
                for r in range(4):
                    p, rh = divmod(r, 2)
                    nc.tensor.transpose(
                        prev_z[p][0:H, rh * 512 + 400 : rh * 512 + 464].bitcast(bf16),
                        HH[p][1 - sb][:, rh * 100 : rh * 100 + 100],
                        id_sb,
                    )
                for p in range(2):
                    hsrc = (
                        prev_z[p][0:H, :]
                        .bitcast(bf16)
                        .rearrange("q (r c) -> q r c", r=2)[:, :, 800:928]
                    )
                    nc.vector.tensor_copy(
                        HT[p][0:H, :].rearrange("q (r c) -> q r c", r=2), hsrc
                    )
            for r in range(4):
                p, rh = divmod(r, 2)
                nc.tensor.matmul(
                    z[p][:, rh * 512 : rh * 512 + G4],
                    HT[p][0:H, rh * 128 : rh * 128 + 128],
                    wh_sb[0:H, r * G4 : (r + 1) * G4],
                    start=False,
                    stop=True,
                )

            # one sigmoid covers all 4 gates (j cols pre-doubled in W)
            for p in range(2):
                z3 = z[p][:, :].rearrange("b (r c) -> b r c", r=2)
                s3 = S[p][sb].rearrange("b (r g) -> b r g", r=2)
                nc.scalar.activation(s3, z3[:, :, 0:G4], Sig)
            for p in range(2):
                s3 = S[p][sb].rearrange("b (r g) -> b r g", r=2)
                pp3 = PP[p].rearrange("b (r g) -> b r g", r=2)
                ff3 = FF[p].rearrange("b (r g) -> b r g", r=2)
                cc3 = CC[p].rearrange("b (r g) -> b r g", r=2)
                # P' = (sig(2zj) - 0.5) * sig(zi)
                nc.vector.scalar_tensor_tensor(
                    pp3, s3[:, :, 300:400], -0.5, s3[:, :, 0:100], add, mult
                )
                # F = sig(zf) * c'
                nc.vector.tensor_tensor(ff3, s3[:, :, 100:200], cc3, mult)
            for p in range(2):
                # c2' = F + P'
                nc.vector.tensor_tensor(CC[p], FF[p], PP[p], add)

            if t == n_steps - 1:
                for p in range(2):
                    nc.sync.dma_start(out[:, p * 200 : (p + 1) * 200], CC[p])
                break

            for p in range(2):
                # tc = tanh(2*c') = tanh(c)
                nc.scalar.activation(TC[p], CC[p], Tanh, scale=2.0)
            for p in range(2):
                s3 = S[p][sb].rearrange("b (r g) -> b r g", r=2)
                tc3 = TC[p].rearrange("b (r g) -> b r g", r=2)
                hh3 = HH[p][sb].rearrange("b (r g) -> b r g", r=2)
                nc.vector.tensor_tensor(hh3, tc3, s3[:, :, 200:300], mult)
            prev_z = z

    nc.compile()
    return nc


def _prep_xt(x_slice):
    """[BL, T, D] fp32 -> [T//TB, 101, TB*3*BL] bf16 block-major tiles.

    tile[n, p, (tq, j, b)] = x[b, n*TB+tq, j*100+p] for p<100; p=100 is the
    baked-in ones row (bias trick).  Each DMA block is a plain 2D copy with
    TB*3*BL*2 contiguous bytes per partition.
    """
    import ml_dtypes

    a = x_slice.transpose(1, 2, 0).reshape(T // TB, TB, 3, 100, BL)
    a = a.transpose(0, 3, 1, 2, 4)  # [n, p, tq, j, b]
    outp = np.zeros((T // TB, KCH, TB, 3, BL), ml_dtypes.bfloat16)
    outp[:, :100] = a.astype(ml_dtypes.bfloat16)
    outp[:, 100] = 1.0
    return outp.reshape(T // TB, KCH, TB * 3 * BL)


def _prep_weights(Ws, bs):
    """Pack 4 runs' [D+H, 4H] weights into [128, 16*400] chunk blocks.

    Gate columns permuted (i,j,f,o) -> (i,f,o,j); the j block (cols 300:400
    after the permute) is scaled by 2 so tanh(zj) = 2*sigmoid(2 zj) - 1 comes
    out of one sigmoid pass.  Chunk-2's row 100 carries the permuted bias
    (+1.0 forget bias on the f block, x2 on the j block).  Also emits the
    recurrent rows (300:400) as bf16 [128, 4*400].
    """
    import ml_dtypes

    perm = np.concatenate(
        [np.arange(0, 100), np.arange(200, 300), np.arange(300, 400), np.arange(100, 200)]
    )
    w_all = np.zeros((KCH, 16 * G4), ml_dtypes.bfloat16)
    wh_bf = np.zeros((128, 4 * G4), ml_dtypes.bfloat16)
    for u in range(4):
        Wp = np.asarray(Ws[u], np.float32)[:, perm].copy()
        Wp[:, 300:400] *= 2.0
        bp = np.asarray(bs[u], np.float32)[perm].copy()
        bp[300:400] *= 2.0
        for k in range(3):
            blk = w_all[:, (u * 4 + k) * G4 : (u * 4 + k + 1) * G4]
            blk[0:100] = Wp[k * 100 : (k + 1) * 100].astype(ml_dtypes.bfloat16)
        bias_row = bp.copy()
        bias_row[100:200] += FORGET_BIAS
        w_all[100, (u * 4 + 2) * G4 : (u * 4 + 3) * G4] = bias_row.astype(
            ml_dtypes.bfloat16
        )
        wh_bf[0:H, u * G4 : (u + 1) * G4] = Wp[300:400].astype(ml_dtypes.bfloat16)
    return w_all, wh_bf


def kernel(premises, hypotheses, Wp_fw, bp_fw, Wp_bw, bp_bw, Wh_fw, bh_fw, Wh_bw, bh_bw):
    from concourse.bass_utils import run_bass_kernel_spmd

    if "nc" not in _CACHE:
        _CACHE["nc"] = _build_program()
    nc = _CACHE["nc"]

    w_all, wh_bf = _prep_weights(
        [Wp_fw, Wp_bw, Wh_fw, Wh_bw], [bp_fw, bp_bw, bh_fw, bh_bw]
    )
    import ml_dtypes

    ident = np.eye(128, dtype=ml_dtypes.bfloat16)

    in_maps = []
    for c in range(NCORES):
        sl = slice(c * BL, (c + 1) * BL)
        in_maps.append(
            {
                "xt_p": _prep_xt(np.asarray(premises[sl], np.float32)),
                "xt_h": _prep_xt(np.asarray(hypotheses[sl], np.float32)),
                "w_all": w_all,
                "wh_bf": wh_bf,
                "ident": ident,
            }
        )

    res = run_bass_kernel_spmd(nc, in_maps, core_ids=list(range(NCORES)))
    # columns are (c_pf, c_pb, c_hf, c_hb) in run order already; state is c/2
    out = np.concatenate([r["out"] for r in res.results], axis=0)
    return 2.0 * out

